# revision 27
# baseline (speedup 1.0000x reference)
"""Cascading sparse attention (GQA decode) on 8 Trainium2 NeuronCores.

Sharding: tensor-parallel over heads. Core c owns q-heads 4c..4c+3 and
kv-head c (Wq/Wk/Wv column slices, Wo row slice, k/v_cache head slice).
Each core computes a partial output (16, 4096); host sums the 8 partials.

Memory-regime design (v2):
  * The position-dependent cascading gather is folded into host-side input
    sharding: K arrives pre-transposed [d, slot] and V slot-major, both
    fp16, densely packed into 2176 slots = n_far + 4 sink + 1 new +
    512 mid + 512 recent + dead. Every cache DMA is then >=4KB-contiguous
    per partition at full HBM bandwidth, with no on-chip K transposes.
  * All weights stream in fp16 (fp32 PSUM accumulation).
  * Logits are computed transposed (out[slot, head] = K_tile^T q) so the
    272 piece outputs pack column-wise into 3 PSUM banks whose layout is
    exactly the attention lhsT layout pT[slot, 64u+4b+h]; the softmax exp
    doubles as the PSUM->SBUF move. Slot-padding / duplicate-row
    corrections collapse to one multiplicative fixup row and one memset;
    the softmax denominator comes from ones-vector matmuls.
"""

import functools
import math
import sys
from collections import Counter
from contextlib import ExitStack

import numpy as np

sys.path.insert(0, "/opt/trn_rl_repo")

import concourse.bass as bass  # noqa: E402
import concourse.bacc as bacc  # noqa: E402
import concourse.tile as tile  # noqa: E402
from concourse import mybir  # noqa: E402
from concourse import masks  # noqa: E402
from concourse import bass_utils  # noqa: E402

F32 = mybir.dt.float32
F16 = mybir.dt.float16
NPF16 = np.float16

SINK, RECENT, MID_W, MID_S, FAR_W, FAR_S = 4, 512, 512, 2, 1536, 4
MAX_CTX = 8192
LN_EPS = 1e-5

B = 16
HID = 4096
H, HKV, D = 32, 8, 128
NCORES = 8
HL = H // NCORES          # 4 local q heads
NBH = HL * B              # 64 (batch, head) pairs
NT = 17                   # slot tiles of 128
SP = NT * 128             # 2176 packed slots
NQKV = HL * D + 2 * D     # 768 fused q|k|v columns
QOFF, KOFF, VOFF = 0, HL * D, HL * D + D
SHIFT = 6.0               # softmax shift; exp(s-6) stays in fp16 range
SCALE = 1.0 / math.sqrt(D)
PAIRS = B // 2


def build_gather_indices(position: int) -> np.ndarray:
    L = position + 1
    idxs = list(range(min(SINK, L))) + [0] * max(0, SINK - L)
    recent_start = max(SINK, L - RECENT)
    r = list(range(recent_start, L))
    while len(r) < RECENT:
        r.insert(0, recent_start)
    idxs += r[-RECENT:]
    mid_end = recent_start
    mid_start = max(SINK, mid_end - MID_W * MID_S)
    m = list(range(mid_start, mid_end, MID_S))
    while len(m) < MID_W:
        m.insert(0, mid_start)
    idxs += m[-MID_W:]
    far_end = mid_start
    far_start = max(SINK, far_end - FAR_W * FAR_S)
    f = list(range(far_start, far_end, FAR_S))
    while len(f) < FAR_W:
        f.insert(0, far_start)
    idxs += f[-FAR_W:]
    return np.asarray(idxs, dtype=np.int64)


@functools.lru_cache(maxsize=4)
def _plan(position: int):
    """Slot layout: [far(n_far), sink(4), new(1), mid(512), recent(512), dead].

    Returns (new_slot, dead_start, rows, w_dup): rows maps slot -> cache
    row (-2 = new token, -1 = dead), w_dup is the multiplicity of the
    far_start row in the reference's padded gather.
    """
    L = position + 1
    recent_start = max(SINK, L - RECENT)
    mid_start = max(SINK, recent_start - MID_W * MID_S)
    far_start = max(SINK, mid_start - FAR_W * FAR_S)
    n_rec = L - recent_start
    n_mid = (recent_start - mid_start + MID_S - 1) // MID_S
    n_far = (mid_start - far_start + FAR_S - 1) // FAR_S
    assert n_rec == RECENT and n_mid == MID_W, "kernel assumes full mid/recent"
    new_slot = n_far + SINK
    dead_start = new_slot + 1 + MID_W + RECENT
    assert dead_start <= SP and dead_start > 16 * 128, "dead must sit in tile 16"

    rows = np.full(SP, -1, dtype=np.int64)
    rows[0:n_far] = far_start + FAR_S * np.arange(n_far)
    rows[n_far:n_far + SINK] = np.arange(SINK)
    rows[new_slot] = -2
    m0 = new_slot + 1
    rows[m0:m0 + MID_W] = mid_start + MID_S * np.arange(MID_W)
    rows[m0 + MID_W:dead_start] = recent_start + np.arange(RECENT)

    counts = Counter(build_gather_indices(position).tolist())
    got = Counter(rows[rows >= 0].tolist())
    assert set(got) == set(counts), "slot map does not cover reference rows"
    assert all(v == 1 for v in got.values()), "duplicate slots for a row"
    extra = {r for r, c in counts.items() if c > 1}
    assert extra <= {far_start}, "only the far_start row may repeat"
    assert rows[0] == far_start
    return new_slot, dead_start, int(counts[far_start])


@functools.lru_cache(maxsize=4)
def _build_program(new_slot: int, dead_start: int, w_dup: int,
                   repeat: int = 1):
    nc = bacc.Bacc("TRN2", target_bir_lowering=False, debug=False,
                   enable_asserts=False, num_devices=NCORES)

    xt_d = nc.dram_tensor("xt", (128, 32 * B), F16, kind="ExternalInput").ap()
    kct_d = nc.dram_tensor("kct", (B, 128, SP), F16, kind="ExternalInput").ap()
    vcp_d = nc.dram_tensor("vcp", (B, 128, SP), F16, kind="ExternalInput").ap()
    wqkv_d = nc.dram_tensor("wqkv", (32, 128, NQKV), F16,
                            kind="ExternalInput").ap()
    wo_d = nc.dram_tensor("wo", (8, 128, HL * 512), F16,
                          kind="ExternalInput").ap()
    cst_d = nc.dram_tensor("cst", (B, 1408), F32, kind="ExternalInput").ap()
    out_d = nc.dram_tensor("out", (B, HID), F32, kind="ExternalOutput").ap()

    NEW_T, NEW_P = new_slot // 128, new_slot % 128

    with tile.TileContext(nc) as tc, ExitStack() as ctx:
        consts = ctx.enter_context(tc.tile_pool(name="consts", bufs=1))
        persist = ctx.enter_context(tc.tile_pool(name="persist", bufs=1))
        small = ctx.enter_context(tc.tile_pool(name="small", bufs=4))
        wqkvp = ctx.enter_context(tc.tile_pool(name="wqkvp", bufs=4))
        vp = ctx.enter_context(tc.tile_pool(name="vp", bufs=5))
        ocp = ctx.enter_context(tc.tile_pool(name="ocp", bufs=4))
        # PSUM budget (8 banks): logits 3 + proj q 1 + proj kv 1 +
        # transposes/attn 3
        psL = ctx.enter_context(tc.tile_pool(name="psL", bufs=3, space="PSUM"))
        psQ = ctx.enter_context(tc.tile_pool(name="psQ", bufs=1, space="PSUM"))
        psKV = ctx.enter_context(
            tc.tile_pool(name="psKV", bufs=1, space="PSUM"))
        psO = ctx.enter_context(tc.tile_pool(name="psO", bufs=3, space="PSUM"))

        ident = consts.tile([128, 128], F32, tag="ident")
        masks.make_identity(nc, ident[:])
        identb = consts.tile([64, 64], F16, tag="identb")
        masks.make_identity(nc, identb[:])
        onesb = consts.tile([128, 1], F16, tag="onesb")
        nc.vector.memset(onesb, 1.0)
        eps_sb = consts.tile([B, 1], F32, tag="eps")
        nc.vector.memset(eps_sb, LN_EPS)
        shift_sb = consts.tile([128, 1], F32, tag="shift")
        nc.vector.memset(shift_sb, -SHIFT)
        # dead-slot denominator correction (see Phase C)
        deadc_sb = consts.tile([NBH, 1], F32, tag="deadc")
        nc.vector.memset(
            deadc_sb,
            -(SP - dead_start) * float(np.asarray(math.exp(-SHIFT), NPF16)))
        cst = consts.tile([B, 1408], F32, tag="cst")
        nc.scalar.dma_start(out=cst, in_=cst_d)
        cs_sb, sn_sb = cst[:, 0:64], cst[:, 64:128]
        qg_sb, qb_sb = cst[:, 128:640], cst[:, 640:1152]
        kg_sb, kb_sb = cst[:, 1152:1280], cst[:, 1280:1408]

        xt = persist.tile([128, 32 * B], F16, tag="xt")
        qT = persist.tile([128, NBH], F16, tag="qT")
        knewT = persist.tile([128, B], F16, tag="knewT")
        kvbf = persist.tile([B, 2 * D], F16, tag="kvbf")
        kall = persist.tile([128, B * SP], F16, tag="kall")
        pT = persist.tile([128, NT * NBH], F16, tag="pT")
        attnT = persist.tile([128, NBH], F16, tag="attnT")
        attn64 = persist.tile([NBH, D], F16, tag="attn64")
        attn64f = persist.tile([NBH, D], F32, tag="attn64f")
        woall = persist.tile([128, 8 * HL * 512], F16, tag="woall")
        rec = persist.tile([NBH, 1], F32, tag="rec")
        gate = persist.tile([1, 2], F16, tag="gate")
        qkv = persist.tile([B, NQKV], F32, tag="qkv")
        qkv2 = persist.tile([B, NQKV], F32, tag="qkv2")

        def _emit_once():
            # Big streams ride separate DMA queues so the 16 engines work
            # in parallel: K + V + Wo on sync, x/weights/staging on scalar.
            for i in range(8):
                nc.sync.dma_start(
                    out=kall[:, 2 * SP * i:2 * SP * (i + 1)]
                        .rearrange("p (a s) -> p a s", a=2),
                    in_=kct_d[2 * i:2 * i + 2].rearrange("a p s -> p a s"))

            # ---- Phase A: QKV projection + LN + RoPE ---------------------
            nc.scalar.dma_start(out=xt, in_=xt_d)
            ps_q = psQ.tile([B, HL * D], F32, tag="q")
            ps_kv = psKV.tile([B, 2 * D], F32, tag="kv")
            for i in range(8):
                wc = wqkvp.tile([128, 4, NQKV], F16, tag="wqkv")
                # split the weight stream across two queues so the K stream
                # keeps more of the sync queue's share of aggregate bandwidth
                weng = nc.scalar if i % 2 == 0 else nc.gpsimd
                weng.dma_start(
                    out=wc,
                    in_=wqkv_d[4 * i:4 * i + 4].rearrange("a p n -> p a n"))
                for a in range(4):
                    c = 4 * i + a
                    lhsT = xt[:, B * c:B * (c + 1)]
                    st, sp = (c == 0), (c == 31)
                    nc.tensor.matmul(ps_q, lhsT, wc[:, a, 0:HL * D],
                                     start=st, stop=sp)
                    nc.tensor.matmul(ps_kv, lhsT, wc[:, a, HL * D:NQKV],
                                     start=st, stop=sp)
            nc.vector.tensor_copy(out=qkv[:, 0:HL * D], in_=ps_q)
            nc.vector.tensor_copy(out=qkv[:, HL * D:NQKV], in_=ps_kv)

            # per-head layernorm over D
            for j in range(HL + 2):
                blk = qkv[:, D * j:D * (j + 1)]
                st6 = small.tile([B, 6], F32, tag="st6")
                mv = small.tile([B, 2], F32, tag="mv")
                nc.vector.bn_stats(out=st6, in_=blk)
                nc.vector.bn_aggr(out=mv, in_=st6)
                nc.scalar.activation(out=mv[:, 1:2], in_=mv[:, 1:2],
                                     func=mybir.ActivationFunctionType.Sqrt,
                                     bias=eps_sb, scale=1.0)
                nc.vector.reciprocal(out=mv[:, 1:2], in_=mv[:, 1:2])
                nc.vector.tensor_scalar(out=blk, in0=blk,
                                        scalar1=mv[:, 0:1], scalar2=mv[:, 1:2],
                                        op0=mybir.AluOpType.subtract,
                                        op1=mybir.AluOpType.mult)
                if j < HL:
                    g = qg_sb[:, D * j:D * (j + 1)]
                    bta = qb_sb[:, D * j:D * (j + 1)]
                elif j == HL:
                    g, bta = kg_sb, kb_sb
                else:
                    g = bta = None
                if g is not None:
                    nc.vector.tensor_mul(out=blk, in0=blk, in1=g)
                    nc.vector.tensor_add(out=blk, in0=blk, in1=bta)

            # RoPE on q heads + k (not v); write into qkv2
            for j in range(HL + 1):
                x1 = qkv[:, D * j:D * j + 64]
                x2 = qkv[:, D * j + 64:D * (j + 1)]
                o1 = qkv2[:, D * j:D * j + 64]
                o2 = qkv2[:, D * j + 64:D * (j + 1)]
                t1 = small.tile([B, 64], F32, tag="t1")
                t2 = small.tile([B, 64], F32, tag="t2")
                nc.vector.tensor_mul(out=t1, in0=x1, in1=cs_sb)
                nc.vector.tensor_mul(out=t2, in0=x2, in1=sn_sb)
                nc.vector.tensor_mul(out=o2, in0=x2, in1=cs_sb)
                nc.vector.tensor_sub(out=o1, in0=t1, in1=t2)
                nc.vector.tensor_mul(out=t2, in0=x1, in1=sn_sb)
                nc.vector.tensor_add(out=o2, in0=o2, in1=t2)
            nc.vector.tensor_copy(out=qkv2[:, VOFF:VOFF + D],
                                  in_=qkv[:, VOFF:VOFF + D])
            # fold logit scale into q
            nc.scalar.mul(out=qkv2[:, 0:HL * D], in_=qkv2[:, 0:HL * D],
                          mul=SCALE)
            nc.vector.tensor_copy(out=kvbf, in_=qkv2[:, KOFF:NQKV])

            # knewT[d, b] and qT[d, 4b+h] via PE transposes
            pst = psO.tile([128, 512], F32, tag="ab", name="pst")
            nc.tensor.transpose(pst[:, 0:B], qkv2[:, KOFF:KOFF + D],
                                ident[:B, :B])
            for h in range(HL):
                nc.tensor.transpose(pst[:, 64 + B * h:64 + B * (h + 1)],
                                    qkv2[:, D * h:D * (h + 1)], ident[:B, :B])
            nc.vector.tensor_copy(out=knewT, in_=pst[:, 0:B])
            nc.vector.tensor_copy(
                out=qT.rearrange("p (b h) -> p h b", h=HL),
                in_=pst[:, 64:64 + NBH].rearrange("p (h b) -> p h b", b=B))
            # insert k_new as column new_slot of every batch block
            nc.vector.tensor_copy(out=kall[:, new_slot:B * SP:SP], in_=knewT)

            # ---- Phase B: transposed logits ------------------------------
            # piece (u, b) = K_tile^T q_b -> [128 slots, 4 heads] lands at
            # bank[:, 4*(16u+b) % 512]; bank layout == pT layout.
            banks = [psL.tile([128, 512], F32, tag="L", name="bank0"),
                     psL.tile([128, 512], F32, tag="L", name="bank1"),
                     psL.tile([128, 64], F32, tag="L", name="bank2")]
            for u in range(NT):
                for b in range(B):
                    qq = 16 * u + b
                    g, m = qq // 128, qq % 128
                    nc.tensor.matmul(
                        banks[g][:, 4 * m:4 * (m + 1)],
                        kall[:, b * SP + 128 * u:b * SP + 128 * (u + 1)],
                        qT[:, HL * b:HL * (b + 1)],
                        start=True, stop=True)

            # ---- Phase C: softmax (exp is the PSUM->SBUF move) -----------
            nc.scalar.activation(out=pT[:, 0:512], in_=banks[0],
                                 func=mybir.ActivationFunctionType.Exp,
                                 bias=shift_sb, scale=1.0)
            nc.scalar.activation(out=pT[:, 512:1024], in_=banks[1],
                                 func=mybir.ActivationFunctionType.Exp,
                                 bias=shift_sb, scale=1.0)
            nc.scalar.activation(out=pT[:, 1024:1088], in_=banks[2],
                                 func=mybir.ActivationFunctionType.Exp,
                                 bias=shift_sb, scale=1.0)
            # correction: duplicated far_start row (slot 0, tile 0)
            if w_dup > 1:
                nc.scalar.mul(out=pT[0:1, 0:64], in_=pT[0:1, 0:64],
                              mul=float(w_dup))
            # denominator: sums[4b+h] = sum_slots pT -- ones-vector matmuls.
            # Dead slots have exactly-zero K and V columns, so each adds
            # exactly fp16(exp(-SHIFT)) to the sum and nothing to the
            # numerator; subtract that known constant instead of masking.
            sm = psQ.tile([NBH, 1], F32, tag="q")
            for u in range(NT):
                nc.tensor.matmul(sm, pT[:, NBH * u:NBH * (u + 1)], onesb,
                                 start=(u == 0), stop=(u == NT - 1))
            if SP - dead_start:
                nc.scalar.activation(
                    out=sm, in_=sm,
                    func=mybir.ActivationFunctionType.Identity,
                    bias=deadc_sb, scale=1.0)
            nc.vector.reciprocal(out=rec, in_=sm)

            # ---- Phase D: V pairs + attention ----------------------------
            # stage DMAs go on the scalar queue: a stage DMA waits on the
            # pair's attention, and on the sync queue it would head-of-line
            # block the next V-pair transfer behind that compute.
            # a single DGE queue tops out well below aggregate DMA
            # bandwidth, so the V stream is split across the sync and
            # gpsimd queues; the gpsimd half is gated behind the last K
            # block so it cannot steal bandwidth from the K stream.
            nc.gpsimd.dma_start(out=gate, in_=kall[0:1, B * SP - 2:B * SP])
            for i in range(PAIRS):
                vb = vp.tile([128, 2 * SP], F16, tag="vb")
                eng = nc.sync if i % 2 == 0 else nc.gpsimd
                eng.dma_start(
                    out=vb.rearrange("p (a s) -> p a s", a=2),
                    in_=vcp_d[2 * i:2 * i + 2].rearrange("a p s -> p a s"))
                nc.sync.dma_start(
                    out=vb[NEW_P:NEW_P + 1, :]
                        .rearrange("o (a s) -> o a s", a=2)
                        [:, :, 128 * NEW_T:128 * (NEW_T + 1)],
                    in_=kvbf[2 * i:2 * i + 2, D:2 * D])
                ab = psO.tile([HL, 2 * D], F32, tag="ab")
                for a in range(2):
                    b = 2 * i + a
                    for u in range(NT):
                        nc.tensor.matmul(
                            ab[:, D * a:D * (a + 1)],
                            pT[:, NBH * u + HL * b:NBH * u + HL * (b + 1)],
                            vb[:, a * SP + 128 * u:a * SP + 128 * (u + 1)],
                            start=(u == 0), stop=(u == NT - 1))
                # compute engines need 32-aligned partition bases, so stage
                # the pair at base 0 and let DMAs place the row blocks
                stg = small.tile([HL, 2 * D], F32, tag="stg")
                nc.vector.tensor_copy(out=stg, in_=ab)
                for a in range(2):
                    b = 2 * i + a
                    nc.scalar.dma_start(
                        out=attn64f[HL * b:HL * (b + 1), :],
                        in_=stg[:, D * a:D * (a + 1)])
            nc.vector.tensor_scalar_mul(out=attn64, in0=attn64f, scalar1=rec)
            psa = psO.tile([128, 512], F16, tag="ab", name="psa")
            nc.tensor.transpose(psa[:, 0:NBH], attn64, identb)
            nc.vector.tensor_copy(out=attnT, in_=psa[:, 0:NBH])

            # Wo stream: emitted here on the sync queue so its transfers
            # follow the V stream; phase E matmuls depend per-chunk via AP
            # overlap, so chunk n starts as soon as its DMA lands.
            for i in range(4):
                eng = nc.sync if i % 2 == 0 else nc.gpsimd
                eng.dma_start(
                    out=woall[:, 4096 * i:4096 * (i + 1)]
                        .rearrange("p (a m) -> p a m", a=2),
                    in_=wo_d[2 * i:2 * i + 2].rearrange("a p m -> p a m"))

            # ---- Phase E: output projection ------------------------------
            for n2 in range(4):
                psWa = psQ.tile([B, 512], F32, tag="q", name="psWa")
                psWb = psKV.tile([B, 512], F32, tag="kv", name="psWb")
                for k in range(HL):
                    for n, psW in ((2 * n2, psWa), (2 * n2 + 1, psWb)):
                        nc.tensor.matmul(
                            psW, attnT[:, k:NBH:HL],
                            woall[:, 2048 * n + 512 * k:2048 * n + 512 * (k + 1)],
                            start=(k == 0), stop=(k == HL - 1))
                for n, psW in ((2 * n2, psWa), (2 * n2 + 1, psWb)):
                    oc = ocp.tile([B, 512], F32, tag="oc")
                    nc.scalar.copy(out=oc, in_=psW)
                    nc.sync.dma_start(out=out_d[:, 512 * n:512 * (n + 1)],
                                      in_=oc)

        for _rep in range(repeat):
            _emit_once()

    nc.compile()
    return nc


def _pack_inputs(inputs):
    """Host-side shard + gather + pack. Returns (in_maps, plan)."""
    hidden = np.asarray(inputs["hidden_states"], dtype=np.float32)
    k_cache = np.asarray(inputs["k_cache"], dtype=np.float32)
    v_cache = np.asarray(inputs["v_cache"], dtype=np.float32)
    position = int(np.asarray(inputs["position"]))
    rope_cos = np.asarray(inputs["rope_cos"], dtype=np.float32)
    rope_sin = np.asarray(inputs["rope_sin"], dtype=np.float32)
    Wq = np.asarray(inputs["Wq"], dtype=np.float32)
    Wk = np.asarray(inputs["Wk"], dtype=np.float32)
    Wv = np.asarray(inputs["Wv"], dtype=np.float32)
    Wo = np.asarray(inputs["Wo"], dtype=np.float32)
    q_gamma = np.asarray(inputs["q_gamma"], dtype=np.float32)
    q_beta = np.asarray(inputs["q_beta"], dtype=np.float32)
    k_gamma = np.asarray(inputs["k_gamma"], dtype=np.float32)
    k_beta = np.asarray(inputs["k_beta"], dtype=np.float32)

    plan = _plan(position)
    new_slot, dead_start, w_dup = plan
    rows = _plan_rows(position)
    rows_clip = np.where(rows >= 0, rows, 0)
    zero_mask = rows < 0

    x = hidden.reshape(B, HID)
    xt = x.T.reshape(32, 128, B).transpose(1, 0, 2).reshape(
        128, 32 * B).astype(NPF16)
    cst = np.zeros((B, 1408), np.float32)
    cst[:, 0:64] = rope_cos[position]
    cst[:, 64:128] = rope_sin[position]
    cst[:, 128:640] = np.tile(q_gamma, HL)
    cst[:, 640:1152] = np.tile(q_beta, HL)
    cst[:, 1152:1280] = k_gamma
    cst[:, 1280:1408] = k_beta

    in_maps = []
    for c in range(NCORES):
        kg_ = k_cache[:, c][:, rows_clip, :]          # (B, SP, D) copy
        kg_[:, zero_mask, :] = 0.0
        kct = kg_.transpose(0, 2, 1).astype(NPF16)   # (B, D, SP)
        vg_ = v_cache[:, c][:, rows_clip, :]
        vg_[:, zero_mask, :] = 0.0
        vcp = vg_.reshape(B, NT, 128, D).transpose(0, 2, 1, 3).reshape(
            B, 128, SP).astype(NPF16)
        wqkv = np.concatenate(
            [Wq[:, c * HL * D:(c + 1) * HL * D],
             Wk[:, c * D:(c + 1) * D],
             Wv[:, c * D:(c + 1) * D]], axis=1).reshape(
                 32, 128, NQKV).astype(NPF16)
        wo_r = Wo[c * HL * D:(c + 1) * HL * D, :].reshape(
            HL, 128, 8, 512).transpose(2, 1, 0, 3).reshape(
                8, 128, HL * 512).astype(NPF16)
        in_maps.append({"xt": xt, "kct": kct, "vcp": vcp,
                        "wqkv": wqkv, "wo": wo_r, "cst": cst})
    return in_maps, plan


@functools.lru_cache(maxsize=4)
def _plan_rows(position: int) -> np.ndarray:
    L = position + 1
    recent_start = max(SINK, L - RECENT)
    mid_start = max(SINK, recent_start - MID_W * MID_S)
    far_start = max(SINK, mid_start - FAR_W * FAR_S)
    n_far = (mid_start - far_start + FAR_S - 1) // FAR_S
    new_slot = n_far + SINK
    rows = np.full(SP, -1, dtype=np.int64)
    rows[0:n_far] = far_start + FAR_S * np.arange(n_far)
    rows[n_far:n_far + SINK] = np.arange(SINK)
    rows[new_slot] = -2
    m0 = new_slot + 1
    rows[m0:m0 + MID_W] = mid_start + MID_S * np.arange(MID_W)
    rows[m0 + MID_W:m0 + MID_W + RECENT] = recent_start + np.arange(RECENT)
    return rows


def kernel(**inputs):
    in_maps, plan = _pack_inputs(inputs)
    new_slot, dead_start, w_dup = plan
    nc = _build_program(new_slot, dead_start, w_dup)
    global _LAST_IN_MAPS
    _LAST_IN_MAPS = in_maps
    res = bass_utils.run_bass_kernel_spmd(
        nc, in_maps, core_ids=list(range(NCORES)))
    global LAST_RESULT
    LAST_RESULT = res
    out = np.zeros((B, HID), dtype=np.float32)
    for r in res.results:
        out += r["out"]
    return out.reshape(B, 1, HID)


LAST_RESULT = None


def timeline_ns(position: int = 6000, trace_path: str | None = None) -> float:
    """Cost-model timeline estimate for one core (no hardware)."""
    from concourse.timeline_sim import TimelineSim

    new_slot, dead_start, w_dup = _plan(position)
    nc = _build_program(new_slot, dead_start, w_dup)
    try:
        ts = TimelineSim(nc, trace=trace_path is not None)
    except AttributeError:
        ts = TimelineSim(nc, trace=False)
        trace_path = None
    t = ts.simulate()
    if trace_path is not None and ts.perfetto is not None:
        ts.perfetto.save(trace_path)
    return t


def bench_hw(inputs, iters: int = 10):
    """On-device kernel time via repeat-variant NEFFs.

    Builds the same program with the body emitted once and R times;
    the difference of their per-dispatch wall times isolates pure
    device execution from the (large) axon dispatch overhead.
    """
    import jax
    from jax.sharding import Mesh, NamedSharding, PartitionSpec
    from jax.experimental.shard_map import shard_map

    import concourse.bass2jax as b2j
    from concourse import mybir as mb

    out = kernel(**inputs)  # noqa: F841  (prepares _LAST_IN_MAPS)
    new_slot, dead_start, w_dup = _plan(int(np.asarray(inputs["position"])))
    in_maps = _LAST_IN_MAPS
    b2j.install_neuronx_cc_hook()
    devices = jax.devices()[:NCORES]
    mesh = Mesh(np.asarray(devices), ("core",))
    spec = PartitionSpec("core")
    sharding = NamedSharding(mesh, spec)

    def make_runner(nc):
        partition_name = (nc.partition_id_tensor.name
                          if nc.partition_id_tensor else None)
        in_names, out_names, out_avals, zero_outs = [], [], [], []
        for alloc in nc.m.functions[0].allocations:
            if not isinstance(alloc, mb.MemoryLocationSet):
                continue
            name = alloc.memorylocations[0].name
            if alloc.kind == "ExternalInput":
                if name != partition_name:
                    in_names.append(name)
            elif alloc.kind == "ExternalOutput":
                out_names.append(name)
                shape = tuple(alloc.tensor_shape)
                dtype = mb.dt.np(alloc.dtype)
                out_avals.append(jax.core.ShapedArray(shape, dtype))
                zero_outs.append(np.zeros(shape, dtype))
        n_params = len(in_names)
        all_names = in_names + out_names
        if partition_name is not None:
            all_names = all_names + [partition_name]
        n_out = len(out_names)

        def _body(*args):
            operands = list(args)
            if partition_name is not None:
                operands.append(b2j.partition_id_tensor())
            outs = b2j._bass_exec_p.bind(
                *operands,
                out_avals=tuple(out_avals),
                in_names=tuple(all_names),
                out_names=tuple(out_names),
                lowering_input_output_aliases=(),
                sim_require_finite=True,
                sim_require_nnan=True,
                nc=nc,
            )
            return tuple(outs)

        fn = jax.jit(
            shard_map(_body, mesh=mesh,
                      in_specs=(spec,) * (n_params + n_out),
                      out_specs=(spec,) * n_out, check_rep=False),
            keep_unused=True,
        )
        concat_in = [
            np.concatenate(
                [np.asarray(in_maps[c][nm]) for c in range(NCORES)], 0)
            for nm in in_names
        ]
        concat_zero = [
            np.zeros((NCORES * z.shape[0], *z.shape[1:]), z.dtype)
            for z in zero_outs
        ]
        dev_in = [jax.device_put(a, sharding) for a in concat_in]
        dev_zero = [jax.device_put(a, sharding) for a in concat_zero]
        jax.block_until_ready(dev_in)

        def run():
            # dispatch a batch of executions before blocking: the on-device
            # signal scales with the batch while the host round-trip
            # overhead pipelines, so the rep-slope SNR improves ~4x
            rs = [fn(*dev_in, *dev_zero) for _ in range(4)]
            jax.block_until_ready(rs)
        return run

    R0, R1 = 4, 40
    r1 = make_runner(_build_program(new_slot, dead_start, w_dup, R0))
    rR = make_runner(_build_program(new_slot, dead_start, w_dup, R1))
    r1(); r1()
    rR(); rR()
    # interleave the two variants so dispatch-time drift cancels in the
    # per-round slope; median over rounds rejects outliers
    iters = max(iters, 24)
    ts1, tsR = [], []
    for _ in range(iters):
        ts1.append(_timed(r1))
        tsR.append(_timed(rR))
    diffs = sorted((b - a) / (4 * (R1 - R0)) for a, b in zip(ts1, tsR))
    n = len(diffs)
    kernel_s = diffs[n // 2] if n % 2 else (diffs[n//2 - 1] + diffs[n//2]) / 2
    print('  raw r%d: %s' % (R0, ' '.join('%.1fms' % (x * 1e3) for x in ts1)))
    print('  raw r%d: %s' % (R1, ' '.join('%.1fms' % (x * 1e3) for x in tsR)))
    print('  per-round slope us: %s' %
          ' '.join('%.1f' % (d * 1e6) for d in diffs))
    return min(ts1), kernel_s


def _timed(f):
    import time
    t0 = time.perf_counter()
    f()
    return time.perf_counter() - t0


_LAST_IN_MAPS = None


# revision 28
# speedup vs baseline: 1.0289x; 1.0289x over previous
"""Cascading sparse attention (GQA decode) on 8 Trainium2 NeuronCores.

Sharding: tensor-parallel over heads. Core c owns q-heads 4c..4c+3 and
kv-head c (Wq/Wk/Wv column slices, Wo row slice, k/v_cache head slice).
Each core computes a partial output (16, 4096); host sums the 8 partials.

Memory-regime design (v2):
  * The position-dependent cascading gather is folded into host-side input
    sharding: K arrives pre-transposed [d, slot] and V slot-major, both
    fp16, densely packed into 2176 slots = n_far + 4 sink + 1 new +
    512 mid + 512 recent + dead. Every cache DMA is then >=4KB-contiguous
    per partition at full HBM bandwidth, with no on-chip K transposes.
  * All weights stream in fp16 (fp32 PSUM accumulation).
  * Logits are computed transposed (out[slot, head] = K_tile^T q) so the
    272 piece outputs pack column-wise into 3 PSUM banks whose layout is
    exactly the attention lhsT layout pT[slot, 64u+4b+h]; the softmax exp
    doubles as the PSUM->SBUF move. Slot-padding / duplicate-row
    corrections collapse to one multiplicative fixup row and one memset;
    the softmax denominator comes from ones-vector matmuls.
"""

import functools
import math
import sys
from collections import Counter
from contextlib import ExitStack

import numpy as np

sys.path.insert(0, "/opt/trn_rl_repo")

import concourse.bass as bass  # noqa: E402
import concourse.bacc as bacc  # noqa: E402
import concourse.tile as tile  # noqa: E402
from concourse import mybir  # noqa: E402
from concourse import masks  # noqa: E402
from concourse import bass_utils  # noqa: E402

F32 = mybir.dt.float32
F16 = mybir.dt.float16
NPF16 = np.float16

SINK, RECENT, MID_W, MID_S, FAR_W, FAR_S = 4, 512, 512, 2, 1536, 4
MAX_CTX = 8192
LN_EPS = 1e-5

B = 16
HID = 4096
H, HKV, D = 32, 8, 128
NCORES = 8
HL = H // NCORES          # 4 local q heads
NBH = HL * B              # 64 (batch, head) pairs
NT = 17                   # slot tiles of 128
SP = NT * 128             # 2176 packed slots
NQKV = HL * D + 2 * D     # 768 fused q|k|v columns
QOFF, KOFF, VOFF = 0, HL * D, HL * D + D
SHIFT = 6.0               # softmax shift; exp(s-6) stays in fp16 range
SCALE = 1.0 / math.sqrt(D)
PAIRS = B // 2


def build_gather_indices(position: int) -> np.ndarray:
    L = position + 1
    idxs = list(range(min(SINK, L))) + [0] * max(0, SINK - L)
    recent_start = max(SINK, L - RECENT)
    r = list(range(recent_start, L))
    while len(r) < RECENT:
        r.insert(0, recent_start)
    idxs += r[-RECENT:]
    mid_end = recent_start
    mid_start = max(SINK, mid_end - MID_W * MID_S)
    m = list(range(mid_start, mid_end, MID_S))
    while len(m) < MID_W:
        m.insert(0, mid_start)
    idxs += m[-MID_W:]
    far_end = mid_start
    far_start = max(SINK, far_end - FAR_W * FAR_S)
    f = list(range(far_start, far_end, FAR_S))
    while len(f) < FAR_W:
        f.insert(0, far_start)
    idxs += f[-FAR_W:]
    return np.asarray(idxs, dtype=np.int64)


@functools.lru_cache(maxsize=4)
def _plan(position: int):
    """Slot layout: [far(n_far), sink(4), new(1), mid(512), recent(512), dead].

    Returns (new_slot, dead_start, rows, w_dup): rows maps slot -> cache
    row (-2 = new token, -1 = dead), w_dup is the multiplicity of the
    far_start row in the reference's padded gather.
    """
    L = position + 1
    recent_start = max(SINK, L - RECENT)
    mid_start = max(SINK, recent_start - MID_W * MID_S)
    far_start = max(SINK, mid_start - FAR_W * FAR_S)
    n_rec = L - recent_start
    n_mid = (recent_start - mid_start + MID_S - 1) // MID_S
    n_far = (mid_start - far_start + FAR_S - 1) // FAR_S
    assert n_rec == RECENT and n_mid == MID_W, "kernel assumes full mid/recent"
    new_slot = n_far + SINK
    dead_start = new_slot + 1 + MID_W + RECENT
    assert dead_start <= SP and dead_start > 16 * 128, "dead must sit in tile 16"

    rows = np.full(SP, -1, dtype=np.int64)
    rows[0:n_far] = far_start + FAR_S * np.arange(n_far)
    rows[n_far:n_far + SINK] = np.arange(SINK)
    rows[new_slot] = -2
    m0 = new_slot + 1
    rows[m0:m0 + MID_W] = mid_start + MID_S * np.arange(MID_W)
    rows[m0 + MID_W:dead_start] = recent_start + np.arange(RECENT)

    counts = Counter(build_gather_indices(position).tolist())
    got = Counter(rows[rows >= 0].tolist())
    assert set(got) == set(counts), "slot map does not cover reference rows"
    assert all(v == 1 for v in got.values()), "duplicate slots for a row"
    extra = {r for r, c in counts.items() if c > 1}
    assert extra <= {far_start}, "only the far_start row may repeat"
    assert rows[0] == far_start
    return new_slot, dead_start, int(counts[far_start])


@functools.lru_cache(maxsize=4)
def _build_program(new_slot: int, dead_start: int, w_dup: int,
                   repeat: int = 1):
    nc = bacc.Bacc("TRN2", target_bir_lowering=False, debug=False,
                   enable_asserts=False, num_devices=NCORES)

    xt_d = nc.dram_tensor("xt", (128, 32 * B), F16, kind="ExternalInput").ap()
    kct_d = nc.dram_tensor("kct", (B, 128, SP), F16, kind="ExternalInput").ap()
    vcp_d = nc.dram_tensor("vcp", (B, 128, SP), F16, kind="ExternalInput").ap()
    wqkv_d = nc.dram_tensor("wqkv", (32, 128, NQKV), F16,
                            kind="ExternalInput").ap()
    wo_d = nc.dram_tensor("wo", (8, 128, HL * 512), F16,
                          kind="ExternalInput").ap()
    cst_d = nc.dram_tensor("cst", (B, 1408), F32, kind="ExternalInput").ap()
    out_d = nc.dram_tensor("out", (B, HID), F32, kind="ExternalOutput").ap()

    NEW_T, NEW_P = new_slot // 128, new_slot % 128

    with tile.TileContext(nc) as tc, ExitStack() as ctx:
        consts = ctx.enter_context(tc.tile_pool(name="consts", bufs=1))
        persist = ctx.enter_context(tc.tile_pool(name="persist", bufs=1))
        small = ctx.enter_context(tc.tile_pool(name="small", bufs=4))
        wqkvp = ctx.enter_context(tc.tile_pool(name="wqkvp", bufs=4))
        vp = ctx.enter_context(tc.tile_pool(name="vp", bufs=6))
        ocp = ctx.enter_context(tc.tile_pool(name="ocp", bufs=4))
        # PSUM budget (8 banks): logits 3 + proj q 1 + proj kv 1 +
        # transposes/attn 3
        psL = ctx.enter_context(tc.tile_pool(name="psL", bufs=3, space="PSUM"))
        psQ = ctx.enter_context(tc.tile_pool(name="psQ", bufs=1, space="PSUM"))
        psKV = ctx.enter_context(
            tc.tile_pool(name="psKV", bufs=1, space="PSUM"))
        psO = ctx.enter_context(tc.tile_pool(name="psO", bufs=3, space="PSUM"))

        ident = consts.tile([128, 128], F32, tag="ident")
        masks.make_identity(nc, ident[:])
        identb = consts.tile([64, 64], F16, tag="identb")
        masks.make_identity(nc, identb[:])
        onesb = consts.tile([128, 1], F16, tag="onesb")
        nc.vector.memset(onesb, 1.0)
        eps_sb = consts.tile([B, 1], F32, tag="eps")
        nc.vector.memset(eps_sb, LN_EPS)
        shift_sb = consts.tile([128, 1], F32, tag="shift")
        nc.vector.memset(shift_sb, -SHIFT)
        # dead-slot denominator correction (see Phase C)
        deadc_sb = consts.tile([NBH, 1], F32, tag="deadc")
        nc.vector.memset(
            deadc_sb,
            -(SP - dead_start) * float(np.asarray(math.exp(-SHIFT), NPF16)))
        cst = consts.tile([B, 1408], F32, tag="cst")
        nc.scalar.dma_start(out=cst, in_=cst_d)
        cs_sb, sn_sb = cst[:, 0:64], cst[:, 64:128]
        qg_sb, qb_sb = cst[:, 128:640], cst[:, 640:1152]
        kg_sb, kb_sb = cst[:, 1152:1280], cst[:, 1280:1408]

        xt = persist.tile([128, 32 * B], F16, tag="xt")
        qT = persist.tile([128, NBH], F16, tag="qT")
        knewT = persist.tile([128, B], F16, tag="knewT")
        kvbf = persist.tile([B, 2 * D], F16, tag="kvbf")
        kall = persist.tile([128, B * SP], F16, tag="kall")
        pT = persist.tile([128, NT * NBH], F16, tag="pT")
        attnT = persist.tile([128, NBH], F16, tag="attnT")
        attn64 = persist.tile([NBH, D], F16, tag="attn64")
        attn64f = persist.tile([NBH, D], F32, tag="attn64f")
        woall = persist.tile([128, 8 * HL * 512], F16, tag="woall")
        rec = persist.tile([NBH, 1], F32, tag="rec")
        gate = persist.tile([1, 2], F16, tag="gate")
        qkv = persist.tile([B, NQKV], F32, tag="qkv")
        qkv2 = persist.tile([B, NQKV], F32, tag="qkv2")

        def _emit_once():
            # Big streams ride separate DMA queues so the 16 engines work
            # in parallel: K + V + Wo on sync, x/weights/staging on scalar.
            for i in range(8):
                nc.sync.dma_start(
                    out=kall[:, 2 * SP * i:2 * SP * (i + 1)]
                        .rearrange("p (a s) -> p a s", a=2),
                    in_=kct_d[2 * i:2 * i + 2].rearrange("a p s -> p a s"))

            # ---- Phase A: QKV projection + LN + RoPE ---------------------
            nc.scalar.dma_start(out=xt, in_=xt_d)
            ps_q = psQ.tile([B, HL * D], F32, tag="q")
            ps_kv = psKV.tile([B, 2 * D], F32, tag="kv")
            for i in range(8):
                wc = wqkvp.tile([128, 4, NQKV], F16, tag="wqkv")
                # split the weight stream across two queues so the K stream
                # keeps more of the sync queue's share of aggregate bandwidth
                weng = nc.scalar if i % 2 == 0 else nc.gpsimd
                weng.dma_start(
                    out=wc,
                    in_=wqkv_d[4 * i:4 * i + 4].rearrange("a p n -> p a n"))
                for a in range(4):
                    c = 4 * i + a
                    lhsT = xt[:, B * c:B * (c + 1)]
                    st, sp = (c == 0), (c == 31)
                    nc.tensor.matmul(ps_q, lhsT, wc[:, a, 0:HL * D],
                                     start=st, stop=sp)
                    nc.tensor.matmul(ps_kv, lhsT, wc[:, a, HL * D:NQKV],
                                     start=st, stop=sp)
            nc.vector.tensor_copy(out=qkv[:, 0:HL * D], in_=ps_q)
            nc.vector.tensor_copy(out=qkv[:, HL * D:NQKV], in_=ps_kv)

            # per-head layernorm over D
            for j in range(HL + 2):
                blk = qkv[:, D * j:D * (j + 1)]
                st6 = small.tile([B, 6], F32, tag="st6")
                mv = small.tile([B, 2], F32, tag="mv")
                nc.vector.bn_stats(out=st6, in_=blk)
                nc.vector.bn_aggr(out=mv, in_=st6)
                nc.scalar.activation(out=mv[:, 1:2], in_=mv[:, 1:2],
                                     func=mybir.ActivationFunctionType.Sqrt,
                                     bias=eps_sb, scale=1.0)
                nc.vector.reciprocal(out=mv[:, 1:2], in_=mv[:, 1:2])
                nc.vector.tensor_scalar(out=blk, in0=blk,
                                        scalar1=mv[:, 0:1], scalar2=mv[:, 1:2],
                                        op0=mybir.AluOpType.subtract,
                                        op1=mybir.AluOpType.mult)
                if j < HL:
                    g = qg_sb[:, D * j:D * (j + 1)]
                    bta = qb_sb[:, D * j:D * (j + 1)]
                elif j == HL:
                    g, bta = kg_sb, kb_sb
                else:
                    g = bta = None
                if g is not None:
                    nc.vector.tensor_mul(out=blk, in0=blk, in1=g)
                    nc.vector.tensor_add(out=blk, in0=blk, in1=bta)

            # RoPE on q heads + k (not v); write into qkv2
            for j in range(HL + 1):
                x1 = qkv[:, D * j:D * j + 64]
                x2 = qkv[:, D * j + 64:D * (j + 1)]
                o1 = qkv2[:, D * j:D * j + 64]
                o2 = qkv2[:, D * j + 64:D * (j + 1)]
                t1 = small.tile([B, 64], F32, tag="t1")
                t2 = small.tile([B, 64], F32, tag="t2")
                nc.vector.tensor_mul(out=t1, in0=x1, in1=cs_sb)
                nc.vector.tensor_mul(out=t2, in0=x2, in1=sn_sb)
                nc.vector.tensor_mul(out=o2, in0=x2, in1=cs_sb)
                nc.vector.tensor_sub(out=o1, in0=t1, in1=t2)
                nc.vector.tensor_mul(out=t2, in0=x1, in1=sn_sb)
                nc.vector.tensor_add(out=o2, in0=o2, in1=t2)
            nc.vector.tensor_copy(out=qkv2[:, VOFF:VOFF + D],
                                  in_=qkv[:, VOFF:VOFF + D])
            # fold logit scale into q
            nc.scalar.mul(out=qkv2[:, 0:HL * D], in_=qkv2[:, 0:HL * D],
                          mul=SCALE)
            nc.vector.tensor_copy(out=kvbf, in_=qkv2[:, KOFF:NQKV])

            # knewT[d, b] and qT[d, 4b+h] via PE transposes
            pst = psO.tile([128, 512], F32, tag="ab", name="pst")
            nc.tensor.transpose(pst[:, 0:B], qkv2[:, KOFF:KOFF + D],
                                ident[:B, :B])
            for h in range(HL):
                nc.tensor.transpose(pst[:, 64 + B * h:64 + B * (h + 1)],
                                    qkv2[:, D * h:D * (h + 1)], ident[:B, :B])
            nc.vector.tensor_copy(out=knewT, in_=pst[:, 0:B])
            nc.vector.tensor_copy(
                out=qT.rearrange("p (b h) -> p h b", h=HL),
                in_=pst[:, 64:64 + NBH].rearrange("p (h b) -> p h b", b=B))
            # insert k_new as column new_slot of every batch block
            nc.vector.tensor_copy(out=kall[:, new_slot:B * SP:SP], in_=knewT)

            # ---- Phase B: transposed logits ------------------------------
            # piece (u, b) = K_tile^T q_b -> [128 slots, 4 heads] lands at
            # bank[:, 4*(16u+b) % 512]; bank layout == pT layout.
            banks = [psL.tile([128, 512], F32, tag="L", name="bank0"),
                     psL.tile([128, 512], F32, tag="L", name="bank1"),
                     psL.tile([128, 64], F32, tag="L", name="bank2")]
            for u in range(NT):
                for b in range(B):
                    qq = 16 * u + b
                    g, m = qq // 128, qq % 128
                    nc.tensor.matmul(
                        banks[g][:, 4 * m:4 * (m + 1)],
                        kall[:, b * SP + 128 * u:b * SP + 128 * (u + 1)],
                        qT[:, HL * b:HL * (b + 1)],
                        start=True, stop=True)

            # ---- Phase C: softmax (exp is the PSUM->SBUF move) -----------
            nc.scalar.activation(out=pT[:, 0:512], in_=banks[0],
                                 func=mybir.ActivationFunctionType.Exp,
                                 bias=shift_sb, scale=1.0)
            nc.scalar.activation(out=pT[:, 512:1024], in_=banks[1],
                                 func=mybir.ActivationFunctionType.Exp,
                                 bias=shift_sb, scale=1.0)
            nc.scalar.activation(out=pT[:, 1024:1088], in_=banks[2],
                                 func=mybir.ActivationFunctionType.Exp,
                                 bias=shift_sb, scale=1.0)
            # correction: duplicated far_start row (slot 0, tile 0)
            if w_dup > 1:
                nc.scalar.mul(out=pT[0:1, 0:64], in_=pT[0:1, 0:64],
                              mul=float(w_dup))
            # denominator: sums[4b+h] = sum_slots pT -- ones-vector matmuls.
            # Dead slots have exactly-zero K and V columns, so each adds
            # exactly fp16(exp(-SHIFT)) to the sum and nothing to the
            # numerator; subtract that known constant instead of masking.
            sm = psQ.tile([NBH, 1], F32, tag="q")
            for u in range(NT):
                nc.tensor.matmul(sm, pT[:, NBH * u:NBH * (u + 1)], onesb,
                                 start=(u == 0), stop=(u == NT - 1))
            if SP - dead_start:
                nc.scalar.activation(
                    out=sm, in_=sm,
                    func=mybir.ActivationFunctionType.Identity,
                    bias=deadc_sb, scale=1.0)
            nc.vector.reciprocal(out=rec, in_=sm)

            # ---- Phase D: V pairs + attention ----------------------------
            # stage DMAs go on the scalar queue: a stage DMA waits on the
            # pair's attention, and on the sync queue it would head-of-line
            # block the next V-pair transfer behind that compute.
            # a single DGE queue tops out well below aggregate DMA
            # bandwidth, so the V stream is split across the sync and
            # gpsimd queues; the gpsimd half is gated behind the last K
            # block so it cannot steal bandwidth from the K stream.
            nc.gpsimd.dma_start(out=gate, in_=kall[0:1, B * SP - 2:B * SP])
            for i in range(PAIRS):
                vb = vp.tile([128, 2 * SP], F16, tag="vb")
                eng = nc.sync if i % 2 == 0 else nc.gpsimd
                eng.dma_start(
                    out=vb.rearrange("p (a s) -> p a s", a=2),
                    in_=vcp_d[2 * i:2 * i + 2].rearrange("a p s -> p a s"))
                nc.sync.dma_start(
                    out=vb[NEW_P:NEW_P + 1, :]
                        .rearrange("o (a s) -> o a s", a=2)
                        [:, :, 128 * NEW_T:128 * (NEW_T + 1)],
                    in_=kvbf[2 * i:2 * i + 2, D:2 * D])
                ab = psO.tile([HL, 2 * D], F32, tag="ab")
                for a in range(2):
                    b = 2 * i + a
                    for u in range(NT):
                        nc.tensor.matmul(
                            ab[:, D * a:D * (a + 1)],
                            pT[:, NBH * u + HL * b:NBH * u + HL * (b + 1)],
                            vb[:, a * SP + 128 * u:a * SP + 128 * (u + 1)],
                            start=(u == 0), stop=(u == NT - 1))
                # compute engines need 32-aligned partition bases, so stage
                # the pair at base 0 and let DMAs place the row blocks
                stg = small.tile([HL, 2 * D], F32, tag="stg")
                nc.vector.tensor_copy(out=stg, in_=ab)
                for a in range(2):
                    b = 2 * i + a
                    nc.scalar.dma_start(
                        out=attn64f[HL * b:HL * (b + 1), :],
                        in_=stg[:, D * a:D * (a + 1)])
            nc.vector.tensor_scalar_mul(out=attn64, in0=attn64f, scalar1=rec)
            psa = psO.tile([128, 512], F16, tag="ab", name="psa")
            nc.tensor.transpose(psa[:, 0:NBH], attn64, identb)
            nc.vector.tensor_copy(out=attnT, in_=psa[:, 0:NBH])

            # Wo stream: emitted here on the sync queue so its transfers
            # follow the V stream; phase E matmuls depend per-chunk via AP
            # overlap, so chunk n starts as soon as its DMA lands.
            for i in range(4):
                eng = nc.sync if i % 2 == 0 else nc.gpsimd
                eng.dma_start(
                    out=woall[:, 4096 * i:4096 * (i + 1)]
                        .rearrange("p (a m) -> p a m", a=2),
                    in_=wo_d[2 * i:2 * i + 2].rearrange("a p m -> p a m"))

            # ---- Phase E: output projection ------------------------------
            for n2 in range(4):
                psWa = psQ.tile([B, 512], F32, tag="q", name="psWa")
                psWb = psKV.tile([B, 512], F32, tag="kv", name="psWb")
                for k in range(HL):
                    for n, psW in ((2 * n2, psWa), (2 * n2 + 1, psWb)):
                        nc.tensor.matmul(
                            psW, attnT[:, k:NBH:HL],
                            woall[:, 2048 * n + 512 * k:2048 * n + 512 * (k + 1)],
                            start=(k == 0), stop=(k == HL - 1))
                for n, psW in ((2 * n2, psWa), (2 * n2 + 1, psWb)):
                    oc = ocp.tile([B, 512], F32, tag="oc")
                    nc.scalar.copy(out=oc, in_=psW)
                    nc.sync.dma_start(out=out_d[:, 512 * n:512 * (n + 1)],
                                      in_=oc)

        for _rep in range(repeat):
            _emit_once()

    nc.compile()
    return nc


def _pack_inputs(inputs):
    """Host-side shard + gather + pack. Returns (in_maps, plan)."""
    hidden = np.asarray(inputs["hidden_states"], dtype=np.float32)
    k_cache = np.asarray(inputs["k_cache"], dtype=np.float32)
    v_cache = np.asarray(inputs["v_cache"], dtype=np.float32)
    position = int(np.asarray(inputs["position"]))
    rope_cos = np.asarray(inputs["rope_cos"], dtype=np.float32)
    rope_sin = np.asarray(inputs["rope_sin"], dtype=np.float32)
    Wq = np.asarray(inputs["Wq"], dtype=np.float32)
    Wk = np.asarray(inputs["Wk"], dtype=np.float32)
    Wv = np.asarray(inputs["Wv"], dtype=np.float32)
    Wo = np.asarray(inputs["Wo"], dtype=np.float32)
    q_gamma = np.asarray(inputs["q_gamma"], dtype=np.float32)
    q_beta = np.asarray(inputs["q_beta"], dtype=np.float32)
    k_gamma = np.asarray(inputs["k_gamma"], dtype=np.float32)
    k_beta = np.asarray(inputs["k_beta"], dtype=np.float32)

    plan = _plan(position)
    new_slot, dead_start, w_dup = plan
    rows = _plan_rows(position)
    rows_clip = np.where(rows >= 0, rows, 0)
    zero_mask = rows < 0

    x = hidden.reshape(B, HID)
    xt = x.T.reshape(32, 128, B).transpose(1, 0, 2).reshape(
        128, 32 * B).astype(NPF16)
    cst = np.zeros((B, 1408), np.float32)
    cst[:, 0:64] = rope_cos[position]
    cst[:, 64:128] = rope_sin[position]
    cst[:, 128:640] = np.tile(q_gamma, HL)
    cst[:, 640:1152] = np.tile(q_beta, HL)
    cst[:, 1152:1280] = k_gamma
    cst[:, 1280:1408] = k_beta

    in_maps = []
    for c in range(NCORES):
        kg_ = k_cache[:, c][:, rows_clip, :]          # (B, SP, D) copy
        kg_[:, zero_mask, :] = 0.0
        kct = kg_.transpose(0, 2, 1).astype(NPF16)   # (B, D, SP)
        vg_ = v_cache[:, c][:, rows_clip, :]
        vg_[:, zero_mask, :] = 0.0
        vcp = vg_.reshape(B, NT, 128, D).transpose(0, 2, 1, 3).reshape(
            B, 128, SP).astype(NPF16)
        wqkv = np.concatenate(
            [Wq[:, c * HL * D:(c + 1) * HL * D],
             Wk[:, c * D:(c + 1) * D],
             Wv[:, c * D:(c + 1) * D]], axis=1).reshape(
                 32, 128, NQKV).astype(NPF16)
        wo_r = Wo[c * HL * D:(c + 1) * HL * D, :].reshape(
            HL, 128, 8, 512).transpose(2, 1, 0, 3).reshape(
                8, 128, HL * 512).astype(NPF16)
        in_maps.append({"xt": xt, "kct": kct, "vcp": vcp,
                        "wqkv": wqkv, "wo": wo_r, "cst": cst})
    return in_maps, plan


@functools.lru_cache(maxsize=4)
def _plan_rows(position: int) -> np.ndarray:
    L = position + 1
    recent_start = max(SINK, L - RECENT)
    mid_start = max(SINK, recent_start - MID_W * MID_S)
    far_start = max(SINK, mid_start - FAR_W * FAR_S)
    n_far = (mid_start - far_start + FAR_S - 1) // FAR_S
    new_slot = n_far + SINK
    rows = np.full(SP, -1, dtype=np.int64)
    rows[0:n_far] = far_start + FAR_S * np.arange(n_far)
    rows[n_far:n_far + SINK] = np.arange(SINK)
    rows[new_slot] = -2
    m0 = new_slot + 1
    rows[m0:m0 + MID_W] = mid_start + MID_S * np.arange(MID_W)
    rows[m0 + MID_W:m0 + MID_W + RECENT] = recent_start + np.arange(RECENT)
    return rows


def kernel(**inputs):
    in_maps, plan = _pack_inputs(inputs)
    new_slot, dead_start, w_dup = plan
    nc = _build_program(new_slot, dead_start, w_dup)
    global _LAST_IN_MAPS
    _LAST_IN_MAPS = in_maps
    res = bass_utils.run_bass_kernel_spmd(
        nc, in_maps, core_ids=list(range(NCORES)))
    global LAST_RESULT
    LAST_RESULT = res
    out = np.zeros((B, HID), dtype=np.float32)
    for r in res.results:
        out += r["out"]
    return out.reshape(B, 1, HID)


LAST_RESULT = None


def timeline_ns(position: int = 6000, trace_path: str | None = None) -> float:
    """Cost-model timeline estimate for one core (no hardware)."""
    from concourse.timeline_sim import TimelineSim

    new_slot, dead_start, w_dup = _plan(position)
    nc = _build_program(new_slot, dead_start, w_dup)
    try:
        ts = TimelineSim(nc, trace=trace_path is not None)
    except AttributeError:
        ts = TimelineSim(nc, trace=False)
        trace_path = None
    t = ts.simulate()
    if trace_path is not None and ts.perfetto is not None:
        ts.perfetto.save(trace_path)
    return t


def bench_hw(inputs, iters: int = 10):
    """On-device kernel time via repeat-variant NEFFs.

    Builds the same program with the body emitted once and R times;
    the difference of their per-dispatch wall times isolates pure
    device execution from the (large) axon dispatch overhead.
    """
    import jax
    from jax.sharding import Mesh, NamedSharding, PartitionSpec
    from jax.experimental.shard_map import shard_map

    import concourse.bass2jax as b2j
    from concourse import mybir as mb

    out = kernel(**inputs)  # noqa: F841  (prepares _LAST_IN_MAPS)
    new_slot, dead_start, w_dup = _plan(int(np.asarray(inputs["position"])))
    in_maps = _LAST_IN_MAPS
    b2j.install_neuronx_cc_hook()
    devices = jax.devices()[:NCORES]
    mesh = Mesh(np.asarray(devices), ("core",))
    spec = PartitionSpec("core")
    sharding = NamedSharding(mesh, spec)

    def make_runner(nc):
        partition_name = (nc.partition_id_tensor.name
                          if nc.partition_id_tensor else None)
        in_names, out_names, out_avals, zero_outs = [], [], [], []
        for alloc in nc.m.functions[0].allocations:
            if not isinstance(alloc, mb.MemoryLocationSet):
                continue
            name = alloc.memorylocations[0].name
            if alloc.kind == "ExternalInput":
                if name != partition_name:
                    in_names.append(name)
            elif alloc.kind == "ExternalOutput":
                out_names.append(name)
                shape = tuple(alloc.tensor_shape)
                dtype = mb.dt.np(alloc.dtype)
                out_avals.append(jax.core.ShapedArray(shape, dtype))
                zero_outs.append(np.zeros(shape, dtype))
        n_params = len(in_names)
        all_names = in_names + out_names
        if partition_name is not None:
            all_names = all_names + [partition_name]
        n_out = len(out_names)

        def _body(*args):
            operands = list(args)
            if partition_name is not None:
                operands.append(b2j.partition_id_tensor())
            outs = b2j._bass_exec_p.bind(
                *operands,
                out_avals=tuple(out_avals),
                in_names=tuple(all_names),
                out_names=tuple(out_names),
                lowering_input_output_aliases=(),
                sim_require_finite=True,
                sim_require_nnan=True,
                nc=nc,
            )
            return tuple(outs)

        fn = jax.jit(
            shard_map(_body, mesh=mesh,
                      in_specs=(spec,) * (n_params + n_out),
                      out_specs=(spec,) * n_out, check_rep=False),
            keep_unused=True,
        )
        concat_in = [
            np.concatenate(
                [np.asarray(in_maps[c][nm]) for c in range(NCORES)], 0)
            for nm in in_names
        ]
        concat_zero = [
            np.zeros((NCORES * z.shape[0], *z.shape[1:]), z.dtype)
            for z in zero_outs
        ]
        dev_in = [jax.device_put(a, sharding) for a in concat_in]
        dev_zero = [jax.device_put(a, sharding) for a in concat_zero]
        jax.block_until_ready(dev_in)

        def run():
            # dispatch a batch of executions before blocking: the on-device
            # signal scales with the batch while the host round-trip
            # overhead pipelines, so the rep-slope SNR improves ~4x
            rs = [fn(*dev_in, *dev_zero) for _ in range(4)]
            jax.block_until_ready(rs)
        return run

    R0, R1 = 4, 40
    r1 = make_runner(_build_program(new_slot, dead_start, w_dup, R0))
    rR = make_runner(_build_program(new_slot, dead_start, w_dup, R1))
    r1(); r1()
    rR(); rR()
    # interleave the two variants so dispatch-time drift cancels in the
    # per-round slope; median over rounds rejects outliers
    iters = max(iters, 24)
    ts1, tsR = [], []
    for _ in range(iters):
        ts1.append(_timed(r1))
        tsR.append(_timed(rR))
    diffs = sorted((b - a) / (4 * (R1 - R0)) for a, b in zip(ts1, tsR))
    n = len(diffs)
    kernel_s = diffs[n // 2] if n % 2 else (diffs[n//2 - 1] + diffs[n//2]) / 2
    print('  raw r%d: %s' % (R0, ' '.join('%.1fms' % (x * 1e3) for x in ts1)))
    print('  raw r%d: %s' % (R1, ' '.join('%.1fms' % (x * 1e3) for x in tsR)))
    print('  per-round slope us: %s' %
          ' '.join('%.1f' % (d * 1e6) for d in diffs))
    return min(ts1), kernel_s


def _timed(f):
    import time
    t0 = time.perf_counter()
    f()
    return time.perf_counter() - t0


_LAST_IN_MAPS = None
